# revision 40
# baseline (speedup 1.0000x reference)
"""Trainium2 Bass kernel for a 16-head decoder self-attention block (v3, fp16).

Reference computation (B=2, S=2048, E=2048, H=16, D=128):
    qkv = X @ W_qkv.T + b_qkv ; RoPE(Q, K) ; attn = softmax(QK^T/sqrt(D) + mask)
    out = (attn @ V reshaped) @ W_o.T + b_o

Sharding over 8 NeuronCores: data parallel over batch (2) x tensor parallel
over 4 head-groups of 4 heads each. Each core computes its group's qkv
projection, attention, and a partial (rank-512) slice of the output
projection; the host sums the 4 partials per batch element.

v3 design vs v2:
  - V is projected directly in [s, d] layout (lhsT = X^T k-slice, moving =
    W_v^T) so the 256 per-head DMA XBAR transposes (80us of queue time and a
    7us PE stall at the phase boundary) disappear entirely.
  - One flat software-pipelined attention stream over (qc, head, ms): the
    exp/AV/denominator machinery crosses group boundaries without draining,
    so the PE never waits for a head transition.
  - The serialized ACT exp stream (1005ns per [128,1024] tile, the phase-2
    floor) gets a head start: ~11 leading score/exp steps are pre-rolled
    into the V-projection phase, which has no ACT work of its own.
  - PSUM is split 4/3/1: scores ring 2x[128,1024], AV accumulators as
    [128,512] halves ring 3 (also reused for the RoPE perm and V psums),
    and a 1-bank denominator slot.
  - Output projection emits immediately after the stream with no barrier.
"""

import contextlib
import math
import sys

import numpy as np

sys.path.insert(0, "/opt/trn_rl_repo")

B, S, E = 2, 2048, 2048
H, D = 16, 128
NCORES = 8
NGROUP = 4          # head groups (tensor parallel)
HPG = H // NGROUP   # heads per group = 4
GE = HPG * D        # group embed width = 512
KT = E // 128       # contraction tiles over E = 16
ST = S // 128       # sequence tiles = 16
SCALE = 1.0 / math.sqrt(D)
NQC = 2             # query chunks of 1024
QW = S // NQC
NGRP = NQC * HPG    # 8 attention (qc, head) groups per core
NSTEP = NGRP * ST   # 128 leading/trailing steps
TRAIL = 3
PREROLL = 14

_CACHE = {}


def _build():
    """Build + compile the per-core Bass program (same program, all cores)."""
    import concourse.bacc as bacc
    import concourse.mybir as mybir
    import concourse.tile as tile

    F32 = mybir.dt.float32
    F16 = mybir.dt.float16
    EXP = mybir.ActivationFunctionType.Exp
    IDENT = mybir.ActivationFunctionType.Identity

    nc = bacc.Bacc("TRN2", target_bir_lowering=False, debug=False)

    # Few, large DMAs: each dma_start costs ~650ns of serial HWDGE issue
    # time, so X/weights/consts are packed into single-trigger tensors.
    xt = nc.dram_tensor("xt", [128, KT, S], F16, kind="ExternalInput").ap()   # [p, k, s] = X[b].T[k*128+p, s]
    w1 = nc.dram_tensor("w1", [8 * 128, E], F16, kind="ExternalInput").ap()   # Q/K m-tiles
    wvt = nc.dram_tensor("wvt", [128, KT * GE], F16, kind="ExternalInput").ap()
    w2 = nc.dram_tensor("w2", [128, KT, GE], F16, kind="ExternalInput").ap()
    cst16 = nc.dram_tensor("cst16", [128, 4352], F16, kind="ExternalInput").ap()  # cos|sin|ones|perm
    cst32 = nc.dram_tensor("cst32", [128, 24], F32, kind="ExternalInput").ap()    # mb|bq
    pout = nc.dram_tensor("pout", [E, S], F16, kind="ExternalOutput").ap()

    with tile.TileContext(nc) as tc, contextlib.ExitStack() as est:
            spool = est.enter_context(tc.tile_pool(name="small", bufs=1))
            qkpool = est.enter_context(tc.tile_pool(name="qk", bufs=1))
            vpool = est.enter_context(tc.tile_pool(name="vsb", bufs=1))
            expool = est.enter_context(tc.tile_pool(name="ex", bufs=18))
            prp = est.enter_context(tc.tile_pool(name="pr", bufs=3))
            accp = est.enter_context(tc.tile_pool(name="acc", bufs=4))
            rcp = est.enter_context(tc.tile_pool(name="rc", bufs=2))
            pss_pool = est.enter_context(tc.tile_pool(name="pss", bufs=2, space="PSUM"))
            psoh_pool = est.enter_context(tc.tile_pool(name="psoh", bufs=3, space="PSUM"))
            psd_pool = est.enter_context(tc.tile_pool(name="psd", bufs=1, space="PSUM"))
            c16_sb = spool.tile([128, 4352], F16, tag="c16")
            c32_sb = spool.tile([128, 24], F32, tag="c32")
            cos_sb = c16_sb[:, 0:2048]
            sin_sb = c16_sb[:, 2048:4096]
            ones_sb = c16_sb[:, 4096:4224]
            perm_sb = c16_sb[:, 4224:4352]
            mb_sb = c32_sb[:, 0:ST]
            bq_sb = c32_sb[:, ST:ST + 8]

            qk_sb = [qkpool.tile([128, S], F16, tag=f"qk{m}", name=f"qk{m}")
                     for m in range(8)]
            v_sb = vpool.tile([128, ST, GE], F16, tag="vsb", name="v_sb")

            # ---- attention stream machinery (closures over shared state) ----
            exs = {}     # leading step -> exp tile
            chain = {}   # group -> running denominator accumulator
            dacc = {}    # group -> final denominator (pre cross-partition)
            rcs = {}     # (group, ns) -> reciprocal tile
            psos = {}    # group -> [pso half ns=0, ns=1]
            at_ref = []  # filled with at_sb once allocated
            p3_ref = {}  # w2s / ost once allocated
            fill_q = []  # output-projection units ready to run as fillers
            fill_mms = []  # pending filler per-MM closures
            filled = set()

            def emit_lead(gl):
                grp, ms = gl // ST, gl % ST
                qc, h = grp // HPG, grp % HPG
                qt, kt_ = qk_sb[2 * h], qk_sb[2 * h + 1]
                ps = pss_pool.tile([128, QW], F32, tag="pss")
                for ns in range(2):
                    nc.tensor.matmul(
                        ps[:, ns * 512:(ns + 1) * 512],
                        kt_[:, ms * 128:(ms + 1) * 128],
                        qt[:, qc * QW + ns * 512: qc * QW + (ns + 1) * 512],
                        start=True, stop=True)
                ex = expool.tile([128, QW], F16, tag="ex")
                nc.scalar.activation(ex[:], ps[:], EXP,
                                     bias=mb_sb[:, ms:ms + 1], scale=SCALE)
                exs[gl] = ex
                if ms % 2 == 1:
                    pr = prp.tile([128, QW], F16, tag="pr")
                    nc.vector.tensor_add(pr[:], exs[gl - 1][:], ex[:])
                    if ms == 1:
                        chain[grp] = pr
                    else:
                        na = accp.tile([128, QW], F16, tag="acc")
                        nc.vector.tensor_add(na[:], chain[grp][:], pr[:])
                        chain[grp] = na
                    if ms == ST - 1:
                        dacc[grp] = chain[grp]

            def emit_trail(gt):
                grp, pms = gt // ST, gt % ST
                qc, h = grp // HPG, grp % HPG
                if pms == 0:
                    psos[grp] = [psoh_pool.tile([128, 512], F32, tag="pso",
                                                name=f"pso{grp}_{ns}")
                                 for ns in range(2)]
                po = psos[grp]
                ex = exs[gt]
                for ns in range(2):
                    nc.tensor.matmul(po[ns][:],
                                     v_sb[:, pms, h * 128:(h + 1) * 128],
                                     ex[:, ns * 512:(ns + 1) * 512],
                                     start=(pms == 0), stop=(pms == ST - 1))
                if pms in (ST - 3, ST - 2):
                    ns = pms - (ST - 3)
                    psd = psd_pool.tile([128, 512], F32, tag="psd")
                    nc.tensor.matmul(psd[:], ones_sb[:],
                                     dacc[grp][:, ns * 512:(ns + 1) * 512],
                                     start=True, stop=True)
                    rc = rcp.tile([128, 512], F32, tag="rc")
                    nc.vector.reciprocal_approx_fast(rc[:], psd[:])
                    rcs[(grp, ns)] = rc
                if pms == ST - 1:
                    at_sb = at_ref[0]
                    for ns in range(2):
                        qsl = slice(qc * QW + ns * 512, qc * QW + (ns + 1) * 512)
                        nc.vector.tensor_mul(at_sb[:, h, qsl], po[ns][:],
                                             rcs[(grp, ns)][:])
                    if grp == HPG - 1:
                        # at[:, :, qc0] complete: qcc=0 o-proj units become
                        # psd-slot fillers for the remaining (qc1) groups
                        fill_q.extend((0, m) for m in range(KT))
                if pms in (6, ST - 1) and not fill_mms:
                    queue_fill_unit()

            def queue_fill_unit():
                # One o-proj m-tile through the 1-bank psd slot (two 512-wide
                # accumulation passes), broken into per-MM closures that the
                # pump loop weaves between leading steps (<=2 per step) so the
                # serialized ACT exp stream is never starved. Absorbs the PE
                # idle the ACT floor would otherwise force.
                if not fill_q:
                    return
                qcc, m = fill_q.pop(0)
                filled.add((qcc, m))
                at_sb = at_ref[0]
                o16 = p3_ref["ost"].tile([128, 1024], F16, tag="o16")
                state = {}

                def mk(ns, j):
                    def emit():
                        if j == 0:
                            state[ns] = psd_pool.tile([128, 512], F32,
                                                      tag="psd",
                                                      name=f"fpsd{m}_{ns}")
                        nc.tensor.matmul(
                            state[ns][:],
                            p3_ref["w2s"][m][:, j * 128:(j + 1) * 128],
                            at_sb[:, j, qcc * 1024 + ns * 512:
                                  qcc * 1024 + (ns + 1) * 512],
                            start=(j == 0), stop=(j == HPG - 1))
                        if j == HPG - 1:
                            nc.vector.tensor_copy(
                                o16[:, ns * 512:(ns + 1) * 512], state[ns][:])
                            if ns == 1:
                                nc.sync.dma_start(
                                    pout[m * 128:(m + 1) * 128,
                                         qcc * 1024:(qcc + 1) * 1024], o16[:])
                    return emit

                fill_mms.extend(mk(ns, j) for ns in range(2)
                                for j in range(HPG))

            lead_i = 0

            # ================= Phase 1: projections =================
            with contextlib.ExitStack() as est1:
                xpool = est1.enter_context(tc.tile_pool(name="xt", bufs=1))
                wvtp = est1.enter_context(tc.tile_pool(name="wvt", bufs=1))
                w1p = est1.enter_context(tc.tile_pool(name="w1p", bufs=2))
                qbp = est1.enter_context(tc.tile_pool(name="qbp", bufs=2))
                rap = est1.enter_context(tc.tile_pool(name="rap", bufs=1))
                stp = est1.enter_context(tc.tile_pool(name="stp", bufs=1))
                # DMA triggers are ~650ns of serial engine time each, so X
                # goes out as a few big chunks. Everything on the critical
                # early path shares the SP rings in strict priority order
                # (per-ring FIFO); consts + the second weight tile ride the
                # otherwise-idle ACT HWDGE rings.
                w1t = [None] * 8
                w1t[0] = w1p.tile([128, E], F16, tag="w", name="w1_0")
                nc.sync.dma_start(w1t[0][:, 0:512], w1[0:128, 0:512])
                xts_all = xpool.tile([128, KT, S], F16, tag="xt", name="xts")
                xts = [xts_all[:, k, :] for k in range(KT)]
                nc.sync.dma_start(xts_all[:, 0:1, 0:1024], xt[:, 0:1, 0:1024])
                nc.sync.dma_start(w1t[0][:, 512:E], w1[0:128, 512:E])
                for ka, kb in ((1, 4), (4, 8), (8, 12), (12, 16)):
                    nc.sync.dma_start(xts_all[:, ka:kb, 0:1024],
                                      xt[:, ka:kb, 0:1024])
                for ka, kb in ((0, 6), (6, 11), (11, 16)):
                    nc.sync.dma_start(xts_all[:, ka:kb, 1024:2048],
                                      xt[:, ka:kb, 1024:2048])
                nc.scalar.dma_start(c32_sb[:], cst32)
                nc.scalar.dma_start(c16_sb[:], cst16)
                w1t[1] = w1p.tile([128, E], F16, tag="w", name="w1_1")
                nc.scalar.dma_start(w1t[1][:], w1[128:256, :])
                wvt_sb = wvtp.tile([128, KT * GE], F16, tag="wvt")

                # PE warmup: dummy matmuls on a zeroed tile run during the
                # otherwise-idle DMA wait, flipping the HAM clock gate to
                # 2.4GHz before the first real matmul arrives.
                warm = qbp.tile([128, 1024], F16, tag="qb", name="warm")
                nc.gpsimd.memset(warm[:], 0)
                wps = pss_pool.tile([128, 1024], F32, tag="pss", name="wps")
                for wi in range(10):
                    nc.tensor.matmul(wps[:, 0:512], warm[:, 0:128],
                                     warm[:, 0:512], start=True, stop=True)

                # ---- Phase 1a: Q/K projections + RoPE ----
                # First four units interleave m0/m1 halves so the X h1 DMA
                # gets one extra unit of slack before its first consumer.
                unit_order = ([(0, 0), (1, 0), (0, 1), (1, 1)]
                              + [(m, h) for m in range(2, 8) for h in range(2)])
                for ui, (m, half) in enumerate(unit_order):
                    if ui % 2 == 1 and ui // 2 + 2 < 8:
                        nw = ui // 2 + 2
                        w1t[nw] = w1p.tile([128, E], F16, tag="w",
                                           name=f"w1_{nw}")
                        nc.sync.dma_start(w1t[nw][:],
                                          w1[nw * 128:(nw + 1) * 128, :])
                    if ui == 4:
                        nc.sync.dma_start(wvt_sb[:], wvt)
                    wt = w1t[m]
                    if True:
                        hs = slice(half * 1024, (half + 1) * 1024)
                        ps = pss_pool.tile([128, 1024], F32, tag="pss")
                        for k in range(KT):
                            for ns in range(2):
                                nc.tensor.matmul(
                                    ps[:, ns * 512:(ns + 1) * 512],
                                    wt[:, k * 128:(k + 1) * 128],
                                    xts[k][:, half * 1024 + ns * 512:
                                            half * 1024 + (ns + 1) * 512],
                                    start=(k == 0), stop=(k == KT - 1))
                        qb = qbp.tile([128, 1024], F16, tag="qb")
                        nc.scalar.activation(qb[:], ps[:], IDENT,
                                             bias=bq_sb[:, m:m + 1], scale=1.0)
                        p2 = []
                        for ns in range(2):
                            t = psoh_pool.tile([128, 512], F32, tag="pso")
                            nc.tensor.matmul(t[:], perm_sb[:],
                                             qb[:, ns * 512:(ns + 1) * 512],
                                             start=True, stop=True)
                            p2.append(t)
                        ra = rap.tile([128, 1024], F16, tag="ra")
                        nc.vector.tensor_mul(ra[:], qb[:], cos_sb[:, hs])
                        st = stp.tile([128, 1024], F16, tag="st")
                        for ns in range(2):
                            ssl = slice(half * 1024 + ns * 512,
                                        half * 1024 + (ns + 1) * 512)
                            nc.vector.tensor_mul(st[:, ns * 512:(ns + 1) * 512],
                                                 p2[ns][:], sin_sb[:, ssl])
                        nc.vector.tensor_add(qk_sb[m][:, hs], st[:], ra[:])

                # ---- Phase 1b: V projection ([s, d] layout) + pre-rolled
                # leading attention steps (scores+exp have no V dependency) ----
                for st_ in range(ST):
                    vp = psoh_pool.tile([128, 512], F32, tag="pso")
                    for k in range(KT):
                        nc.tensor.matmul(vp[:],
                                         xts[k][:, st_ * 128:(st_ + 1) * 128],
                                         wvt_sb[:, k * GE:(k + 1) * GE],
                                         start=(k == 0), stop=(k == KT - 1))
                    nc.vector.tensor_copy(v_sb[:, st_, :], vp[:])
                    if st_ >= 4 and lead_i < PREROLL:
                        emit_lead(lead_i)
                        lead_i += 1

            # xts / wvt / w1 / trig freed here; at_sb + w2 take their place
            with contextlib.ExitStack() as est2:
                atp = est2.enter_context(tc.tile_pool(name="at", bufs=1))
                w2p = est2.enter_context(tc.tile_pool(name="w2p", bufs=1))
                ost = est2.enter_context(tc.tile_pool(name="ost", bufs=4))
                at_sb = atp.tile([128, HPG, S], F16, tag="at", name="at_sb")
                at_ref.append(at_sb)
                w2_all = w2p.tile([128, KT, GE], F16, tag="w2", name="w2_all")
                nc.sync.dma_start(w2_all[:], w2)
                w2s = [w2_all[:, m, :] for m in range(KT)]
                p3_ref["w2s"] = w2s
                p3_ref["ost"] = ost

                # ---- Phase 2: flat attention stream ----
                # Filler cadence ~1 MM per 2 steps matches the ~142ns/step
                # PE deficit vs the serialized ACT exp stream without
                # starving it.
                trail_i = 0
                it = 0
                while trail_i < NSTEP:
                    it += 1
                    if lead_i < NSTEP:
                        emit_lead(lead_i)
                        lead_i += 1
                    budget = 2 if (lead_i - trail_i) > TRAIL + 1 else 1
                    if lead_i >= NSTEP:
                        budget = NSTEP - trail_i
                    for _ in range(budget):
                        if trail_i < NSTEP and trail_i < lead_i:
                            emit_trail(trail_i)
                            trail_i += 1
                    if fill_mms and (it % 2 == 0 or lead_i >= NSTEP):
                        fill_mms.pop(0)()
                while fill_mms:
                    fill_mms.pop(0)()

                # ---- Phase 3: output projection (partial) ----
                for qcc in range(2):
                    for m in range(KT):
                        if (qcc, m) in filled:
                            continue
                        op = pss_pool.tile([128, 1024], F32, tag="pss")
                        for j in range(HPG):
                            for ns in range(2):
                                nc.tensor.matmul(
                                    op[:, ns * 512:(ns + 1) * 512],
                                    w2s[m][:, j * 128:(j + 1) * 128],
                                    at_sb[:, j, qcc * 1024 + ns * 512:
                                          qcc * 1024 + (ns + 1) * 512],
                                    start=(j == 0), stop=(j == HPG - 1))
                        o16 = ost.tile([128, 1024], F16, tag="o16")
                        if (qcc * KT + m) % 2 == 0:
                            nc.scalar.activation(o16[:], op[:], IDENT, scale=1.0)
                        else:
                            nc.vector.tensor_copy(o16[:], op[:])
                        nc.sync.dma_start(
                            pout[m * 128:(m + 1) * 128,
                                 qcc * 1024:(qcc + 1) * 1024], o16[:])

    nc.compile()
    return nc


def _rope_tables():
    # Bug-faithful to the reference: exponent divides by EMB_DIM, not head_dim.
    angle = 1.0 / np.power(10000.0, np.arange(0, D, 2, dtype=np.float64) / E)
    t = np.arange(S, dtype=np.float64)
    freqs = np.repeat(t[:, None] * angle[None, :], 2, axis=-1)  # [S, D]
    return np.cos(freqs), np.sin(freqs)


def _prep_inputs(X, mask, W_qkv, b_qkv, W_o, b_o):
    """Build the 8 per-core input maps."""
    X = np.ascontiguousarray(np.asarray(X, dtype=np.float32))
    mask = np.asarray(mask)
    W_qkv = np.asarray(W_qkv, dtype=np.float32)
    b_qkv = np.asarray(b_qkv, dtype=np.float32)
    W_o = np.asarray(W_o, dtype=np.float32)

    cos, sin = _rope_tables()
    cosx = np.ascontiguousarray(cos.T.astype(np.float16))   # [D, S] fp16
    sinx = np.ascontiguousarray(sin.T.astype(np.float16))   # [D, S] fp16
    ones = np.ones((128, 128), dtype=np.float16)
    # trans(q)[j] = -q[2j+1] (j<64), +q[2j-128] (j>=64), as lhsT: permT[d, j]
    permT = np.zeros((128, 128), dtype=np.float16)
    for j in range(64):
        permT[2 * j + 1, j] = -1.0
    for j in range(64, 128):
        permT[2 * (j - 64), j] = 1.0

    # [p, k, s] = X[b].T[k*128+p, s]
    xts = [np.ascontiguousarray(
        X[b].T.astype(np.float16).reshape(KT, 128, S).transpose(1, 0, 2))
        for b in range(B)]
    cst16 = np.ascontiguousarray(np.hstack([cosx, sinx, ones, permT]))
    mbs = []
    for b in range(B):
        m = np.where(mask[b] == 0, np.float32(-1e9), np.float32(0.0)).astype(np.float32)
        mbs.append(np.ascontiguousarray(m.reshape(ST, 128).T))

    W1T = W_qkv.T                                           # [E, 3E]

    def pack_mtile(row0):
        # [128, E] with [p, k*128+c] = W1T[k*128+p, row0+c]
        blk = W1T[:, row0:row0 + 128].reshape(KT, 128, 128)  # [k, p, c]
        return blk.transpose(1, 0, 2).reshape(128, E).astype(np.float16)

    in_maps = []
    for c in range(NCORES):
        b, g = divmod(c, NGROUP)
        w1_rows = []
        bq_cols = []
        for hh in range(HPG):
            h = g * HPG + hh
            w1_rows.append(pack_mtile(h * D))               # Q_h
            bq_cols.append(b_qkv[h * D:(h + 1) * D])
            w1_rows.append(pack_mtile(E + h * D))           # K_h
            bq_cols.append(b_qkv[E + h * D:E + (h + 1) * D])
        w1p = np.concatenate(w1_rows, axis=0)               # [8*128, E]
        bqp = np.stack(bq_cols, axis=1).astype(np.float32)  # [128, 8]

        # wvt: [p, k*GE + j] = W_v_group[j, k*128+p]
        Wv_g = W_qkv[2 * E + g * GE: 2 * E + (g + 1) * GE, :]   # [512, E]
        wvt_ = Wv_g.reshape(GE, KT, 128).transpose(2, 1, 0).reshape(
            128, KT * GE).astype(np.float16)

        # w2: [p, m, j*128+c] = W_o^T[g*GE + j*128 + p, m*128+c]
        W2T = W_o.T[g * GE:(g + 1) * GE, :]                 # [512, E]
        w2p_ = W2T.reshape(HPG, 128, KT, 128).transpose(1, 2, 0, 3).reshape(
            128, KT, GE).astype(np.float16)

        cst32 = np.ascontiguousarray(
            np.hstack([mbs[b], bqp]).astype(np.float32))

        in_maps.append({
            "xt": xts[b],
            "w1": np.ascontiguousarray(w1p),
            "wvt": np.ascontiguousarray(wvt_),
            "w2": np.ascontiguousarray(w2p_),
            "cst16": cst16,
            "cst32": cst32,
        })
    return in_maps


def kernel(X, mask, W_qkv, b_qkv, W_o, b_o, _trace=False):
    from concourse.bass_utils import run_bass_kernel_spmd

    if "nc" not in _CACHE:
        _CACHE["nc"] = _build()
    nc = _CACHE["nc"]

    in_maps = _prep_inputs(X, mask, W_qkv, b_qkv, W_o, b_o)
    res = run_bass_kernel_spmd(nc, in_maps, core_ids=list(range(NCORES)),
                               trace=_trace)
    _CACHE["last_result"] = res

    W_o = np.asarray(W_o, dtype=np.float32)
    b_o_eff = (np.asarray(b_o, dtype=np.float32)
               + W_o @ np.asarray(b_qkv, dtype=np.float32)[2 * E:])

    out = np.empty((B, S, E), dtype=np.float32)
    for b in range(B):
        acc = res.results[b * NGROUP]["pout"].astype(np.float32)
        for g in range(1, NGROUP):
            acc += res.results[b * NGROUP + g]["pout"].astype(np.float32)
        out[b] = acc.T + b_o_eff
    return out
